# revision 4
# baseline (speedup 1.0000x reference)
"""Trainium2 Bass kernel for nn_LocalModel_76527727280750 (sparse_attention).

8-core SPMD: head-parallel attention (core c owns head c, both batches) +
token-parallel LayerNorm/FFN (core c owns tokens [c*512,(c+1)*512) of the
flattened [B*S] axis). Cross-core resharding via AllToAll (attention out:
head-split -> token-split) and AllGather (next-layer input, transposed bf16).

Self-contained: hardcodes all shapes; host does the embedding gather,
weight slicing/packing, and the tiny final reduction.
"""

import sys

for _p in ("/opt/trn_rl_repo",):
    if _p not in sys.path:
        sys.path.append(_p)

import numpy as np
import ml_dtypes

import concourse.bass as bass  # noqa: F401  (registers types)
import concourse.mybir as mybir
import concourse.tile as tile
from concourse import bacc
from concourse.bass_utils import run_bass_kernel_spmd
from concourse.masks import make_identity

# ---- model dims (hardcoded from the problem spec) ----
NC = 8
B, S, E, H, W, HID, V, OUT, L = 2, 2048, 512, 8, 5, 2048, 32000, 6, 6
DH = E // H            # 64
SCALE = DH ** -0.5     # 0.125
PAD = (W - 1) // 2     # 2
SK = S - W + 1         # 2044
BS = B * S             # 4096
CHUNK = BS // NC       # 512 tokens per core
NT = (SK + 127) // 128  # 16 t-blocks (last = 124 wide)

f32 = mybir.dt.float32
bf16 = mybir.dt.bfloat16
AF = mybir.ActivationFunctionType


def _twidth(tb):
    return min(128, SK - tb * 128)


def build_nc():
    nc = bacc.Bacc("TRN2", target_bir_lowering=False, debug=False,
                   enable_asserts=False, num_devices=NC)

    # ---------------- I/O ----------------
    xsrc0 = nc.dram_tensor("xsrc0", [BS, E], bf16, kind="ExternalInput")
    qkw_d = nc.dram_tensor("qkw", [128, 4, 128], bf16, kind="ExternalInput")
    vw_d = nc.dram_tensor("vw", [128, 4, DH], bf16, kind="ExternalInput")
    qkb_d = nc.dram_tensor("qkb", [128, 1], f32, kind="ExternalInput")
    vb_d = nc.dram_tensor("vb", [DH, 1], f32, kind="ExternalInput")
    fc1w_d = nc.dram_tensor("fc1w", [128, 4, HID], bf16, kind="ExternalInput")
    fc1b_d = nc.dram_tensor("fc1b", [128, HID // 128], f32, kind="ExternalInput")
    fc2w_d = nc.dram_tensor("fc2w", [128, HID // 128, E], bf16, kind="ExternalInput")
    fc2b_d = nc.dram_tensor("fc2b", [E], f32, kind="ExternalInput")
    lnw_d = nc.dram_tensor("lnw", [E], f32, kind="ExternalInput")
    lnb_d = nc.dram_tensor("lnb", [E], f32, kind="ExternalInput")
    outw_d = nc.dram_tensor("outw", [OUT, CHUNK, E], f32, kind="ExternalInput")
    headp_d = nc.dram_tensor("headp", [4, 128, OUT], f32, kind="ExternalOutput")

    with tile.TileContext(nc) as tc:
        with (
            tc.tile_pool(name="const", bufs=1) as cst,
            tc.tile_pool(name="xt", bufs=6) as xt_pool,
            tc.tile_pool(name="stk", bufs=1) as stk,
            tc.tile_pool(name="work", bufs=2) as work,
            tc.tile_pool(name="pt", bufs=4) as ptp,
            tc.tile_pool(name="small", bufs=4) as small,
            tc.tile_pool(name="ps_mm", bufs=3, space="PSUM") as ps_mm,
            tc.tile_pool(name="ps_o", bufs=2, space="PSUM") as ps_o,
            tc.tile_pool(name="ps_tr", bufs=2, space="PSUM") as ps_tr,
            tc.tile_pool(name="dram", bufs=2, space="DRAM") as dram,
        ):
            # -------- persistent constants --------
            qkw_sb = cst.tile([128, 4, 128], bf16)
            nc.sync.dma_start(qkw_sb[:], qkw_d[:])
            vw_sb = cst.tile([128, 4, DH], bf16)
            nc.sync.dma_start(vw_sb[:], vw_d[:])
            qkb_sb = cst.tile([128, 1], f32)
            nc.sync.dma_start(qkb_sb[:], qkb_d[:])
            vb_sb = cst.tile([DH, 1], f32)
            nc.sync.dma_start(vb_sb[:], vb_d[:])
            fc1w_sb = cst.tile([128, 4, HID], bf16)
            nc.sync.dma_start(fc1w_sb[:], fc1w_d[:])
            fc1b_sb = cst.tile([128, HID // 128], f32)
            nc.sync.dma_start(fc1b_sb[:], fc1b_d[:])
            fc2w_sb = cst.tile([128, HID // 128, E], bf16)
            nc.sync.dma_start(fc2w_sb[:], fc2w_d[:])
            fc2b_bc = cst.tile([128, E], f32)
            nc.sync.dma_start(fc2b_bc[:], fc2b_d.ap()[None, :].to_broadcast([128, E]))
            lnw_bc = cst.tile([128, E], f32)
            nc.sync.dma_start(lnw_bc[:], lnw_d.ap()[None, :].to_broadcast([128, E]))
            lnb_bc = cst.tile([128, E], f32)
            nc.sync.dma_start(lnb_bc[:], lnb_d.ap()[None, :].to_broadcast([128, E]))
            ident = cst.tile([128, 128], f32)
            make_identity(nc, ident[:])
            eps_sb = cst.tile([128, 1], f32)
            nc.vector.memset(eps_sb[:], 1e-5)

            xsrc_ap = xsrc0.ap()

            for l in range(L):
                # stacks for both batches of this layer
                # q stacks: rows j (paired) of Q_un^T; col s holds qT[:, s+j-2]
                # k stacks: col t holds kT[:, t+j]
                qs, ks, vaug = [], [], []
                for b in range(B):
                    qs0 = stk.tile([128, S], bf16, tag=f"qs0_{b}")
                    qs1 = stk.tile([128, S], bf16, tag=f"qs1_{b}")
                    qs2 = stk.tile([64, S], bf16, tag=f"qs2_{b}")
                    ks0 = stk.tile([128, S], bf16, tag=f"ks0_{b}")
                    ks1 = stk.tile([128, S], bf16, tag=f"ks1_{b}")
                    ks2 = stk.tile([64, S], bf16, tag=f"ks2_{b}")
                    for t in (qs0, qs1, qs2):
                        nc.vector.memset(t[:], 0.0)
                    qs.append((qs0, qs1, qs2))
                    ks.append((ks0, ks1, ks2))

                    vT = stk.tile([DH, S], f32, tag=f"vT_{b}")

                    # ---- q/k/v projections for this head, all tokens of batch b
                    for g in range(4):
                        cb = b * 4 + g  # global chunk id == s-block of 512 tokens
                        qk_ps = ps_mm.tile([128, 512], f32, tag="mmps")
                        v_ps = ps_mm.tile([128, 512], f32, tag="mmps")
                        for ec in range(4):
                            xt = xt_pool.tile([128, 512], bf16, tag="xt")
                            nc.sync.dma_start(
                                xt[:], xsrc_ap[cb * 512 + ec * 128:cb * 512 + (ec + 1) * 128, :])
                            nc.tensor.matmul(qk_ps[:], qkw_sb[:, ec, :], xt[:],
                                             start=(ec == 0), stop=(ec == 3))
                            nc.tensor.matmul(v_ps[0:DH, :], vw_sb[:, ec, :], xt[:],
                                             start=(ec == 0), stop=(ec == 3))
                        qk_sb = work.tile([128, 512], f32, tag="qksb", bufs=2)
                        nc.scalar.activation(qk_sb[:], qk_ps[:], AF.Identity, bias=qkb_sb[:])
                        nc.scalar.activation(vT[:, g * 512:(g + 1) * 512], v_ps[0:DH, :],
                                             AF.Identity, bias=vb_sb[:])
                        # scatter shifted copies into the stacks
                        # q: dest_col = src_col + (2 - j); k: dest_col = src_col - j
                        qdst = [(qs0, 0, 2), (qs0, 64, 1), (qs1, 0, 0), (qs1, 64, -1),
                                (qs2, 0, -2)]
                        kdst = [(ks0, 0, 0), (ks0, 64, -1), (ks1, 0, -2), (ks1, 64, -3),
                                (ks2, 0, -4)]
                        for (srow, lim, lst) in ((0, S, qdst), (64, SK, kdst)):
                            for (dstt, drow, off) in lst:
                                lo = max(0, g * 512 + off)
                                hi = min(lim, g * 512 + 512 + off)
                                if hi <= lo:
                                    continue
                                nc.any.tensor_copy(
                                    dstt[drow:drow + 64, lo:hi],
                                    qk_sb[srow:srow + 64, lo - off - g * 512:hi - off - g * 512])

                    # ---- windowed v-sum + ones row, transposed into v_aug tiles
                    vs = stk.tile([DH + 1, S], f32, tag=f"vs_{b}")
                    nc.vector.memset(vs[DH:DH + 1, 0:SK], 1.0)
                    nc.vector.tensor_add(vs[0:DH, 0:SK], vT[:, 0:SK], vT[:, 1:SK + 1])
                    nc.vector.tensor_add(vs[0:DH, 0:SK], vs[0:DH, 0:SK], vT[:, 2:SK + 2])
                    nc.vector.tensor_add(vs[0:DH, 0:SK], vs[0:DH, 0:SK], vT[:, 3:SK + 3])
                    nc.vector.tensor_add(vs[0:DH, 0:SK], vs[0:DH, 0:SK], vT[:, 4:SK + 4])

                    va = stk.tile([128, NT, DH + 1], bf16, tag=f"vaug_{b}")
                    for tb in range(NT):
                        tw = _twidth(tb)
                        trp = ps_tr.tile([128, 128], f32, tag="trps")
                        nc.tensor.transpose(trp[0:tw, 0:DH + 1],
                                            vs[:, tb * 128:tb * 128 + tw],
                                            ident[0:DH + 1, 0:DH + 1])
                        nc.any.tensor_copy(va[0:tw, tb, :], trp[0:tw, 0:DH + 1])
                    vaug.append(va)

                # -------- attention (scores^T -> exp -> oT accum) --------
                a2a_in = dram.tile([BS, DH], f32, tag="a2a_in")
                for b in range(B):
                    qs0, qs1, qs2 = qs[b]
                    ks0, ks1, ks2 = ks[b]
                    va = vaug[b]
                    for g in range(4):
                        oT_ps = ps_o.tile([DH + 1, 512], f32, tag="ops")
                        for tb in range(NT):
                            tw = _twidth(tb)
                            s_ps = ps_mm.tile([128, 512], f32, tag="mmps")
                            nc.tensor.matmul(s_ps[0:tw, :],
                                             ks0[:, tb * 128:tb * 128 + tw],
                                             qs0[:, g * 512:(g + 1) * 512],
                                             start=True, stop=False)
                            nc.tensor.matmul(s_ps[0:tw, :],
                                             ks1[:, tb * 128:tb * 128 + tw],
                                             qs1[:, g * 512:(g + 1) * 512],
                                             start=False, stop=False)
                            nc.tensor.matmul(s_ps[0:tw, :],
                                             ks2[:, tb * 128:tb * 128 + tw],
                                             qs2[:, g * 512:(g + 1) * 512],
                                             start=False, stop=True)
                            pt = ptp.tile([128, 512], bf16, tag="pt")
                            nc.scalar.activation(pt[0:tw, :], s_ps[0:tw, :], AF.Exp,
                                                 scale=SCALE)
                            nc.tensor.matmul(oT_ps[:], va[0:tw, tb, :], pt[0:tw, :],
                                             start=(tb == 0), stop=(tb == NT - 1))
                        oT_sb = work.tile([DH + 1, 512], f32, tag="otsb")
                        nc.any.tensor_copy(oT_sb[:], oT_ps[:])
                        for tt in range(4):
                            trp = ps_tr.tile([128, 128], f32, tag="trps")
                            nc.tensor.transpose(trp[0:128, 0:DH + 1],
                                                oT_sb[:, tt * 128:(tt + 1) * 128],
                                                ident[0:DH + 1, 0:DH + 1])
                            rcp = small.tile([128, 1], f32, tag="rcp")
                            nc.vector.reciprocal(rcp[:], trp[:, DH:DH + 1])
                            o_st = small.tile([128, DH], f32, tag="ost")
                            nc.vector.tensor_scalar_mul(o_st[:], trp[:, 0:DH], rcp[:])
                            nc.sync.dma_start(
                                a2a_in[(b * 4 + g) * 512 + tt * 128:
                                       (b * 4 + g) * 512 + (tt + 1) * 128, :],
                                o_st[:])

                # -------- reshard: head-split -> token-split --------
                a2a_out = dram.tile([BS, DH], f32, tag="a2a_out")
                nc.gpsimd.collective_compute(
                    "AllToAll", mybir.AluOpType.bypass,
                    replica_groups=[list(range(NC))],
                    ins=[a2a_in.opt()], outs=[a2a_out.opt()],
                )

                # -------- LN1 + FFN + LN2 on my 512-token chunk --------
                y_all = work.tile([128, 4, E], f32, tag="yall", bufs=1)
                a2a_src = a2a_out[:].rearrange("(i r) d -> r i d", i=NC)  # [512, 8, 64]

                def layer_norm(xap, tt):
                    mneg = small.tile([128, 1], f32, tag="mneg")
                    nc.vector.reduce_sum(mneg[:], xap, axis=mybir.AxisListType.X)
                    nc.vector.tensor_scalar_mul(mneg[:], mneg[:], -1.0 / E)
                    nc.vector.tensor_scalar_add(xap, xap, mneg[:])
                    sq = work.tile([128, E], bf16, tag="sq", bufs=1)
                    ss = small.tile([128, 1], f32, tag="ss")
                    nc.scalar.activation(sq[:], xap, AF.Square, accum_out=ss[:])
                    sd = small.tile([128, 1], f32, tag="sd")
                    nc.scalar.activation(sd[:], ss[:], AF.Sqrt, bias=eps_sb[:], scale=1.0 / E)
                    rs = small.tile([128, 1], f32, tag="rs")
                    nc.vector.reciprocal(rs[:], sd[:])
                    nc.vector.tensor_scalar_mul(xap, xap, rs[:])
                    nc.vector.tensor_mul(xap, xap, lnw_bc[:])
                    nc.vector.tensor_add(xap, xap, lnb_bc[:])

                yT_sb = work.tile([128, 4, 512], bf16, tag="yT", bufs=1)
                for tt in range(4):
                    yv = y_all[:, tt, :]
                    nc.sync.dma_start(
                        yv.rearrange("p (i d) -> p i d", d=DH),
                        a2a_src[tt * 128:(tt + 1) * 128, :, :])
                    layer_norm(yv, tt)
                    for ec in range(4):
                        trp = ps_tr.tile([128, 128], f32, tag="trps")
                        nc.tensor.transpose(trp[:], yv[:, ec * 128:(ec + 1) * 128], ident[:])
                        nc.any.tensor_copy(yT_sb[:, ec, tt * 128:(tt + 1) * 128], trp[:])

                hT_sb = work.tile([128, HID // 128, 512], bf16, tag="hT", bufs=1)
                for hb in range(HID // 128):
                    h_ps = ps_mm.tile([128, 512], f32, tag="mmps")
                    for ec in range(4):
                        nc.tensor.matmul(h_ps[:], fc1w_sb[:, ec, hb * 128:(hb + 1) * 128],
                                         yT_sb[:, ec, :], start=(ec == 0), stop=(ec == 3))
                    nc.scalar.activation(hT_sb[:, hb, :], h_ps[:], AF.Relu,
                                         bias=fc1b_sb[:, hb:hb + 1])

                xn_all = work.tile([128, 4, E], f32, tag="xn", bufs=1)
                if l < L - 1:
                    ag2_in = dram.tile([E, CHUNK], bf16, tag="ag2_in")
                    xTc_sb = work.tile([128, 4, 512], bf16, tag="xTc", bufs=1)
                for tt in range(4):
                    x2_ps = ps_mm.tile([128, 512], f32, tag="mmps")
                    for hc in range(HID // 128):
                        nc.tensor.matmul(x2_ps[:], hT_sb[:, hc, tt * 128:(tt + 1) * 128],
                                         fc2w_sb[:, hc, :],
                                         start=(hc == 0), stop=(hc == HID // 128 - 1))
                    xn = xn_all[:, tt, :]
                    nc.vector.tensor_add(xn, x2_ps[:], y_all[:, tt, :])
                    nc.vector.tensor_add(xn, xn, fc2b_bc[:])
                    layer_norm(xn, tt)
                    if l == L - 1:
                        # final head partials: T[s,o] = sum_e x[s,e] * Wr[o,s,e]
                        acc = small.tile([128, OUT], f32, tag="acc")
                        for o in range(OUT):
                            wro = work.tile([128, E], f32, tag="wro", bufs=2)
                            nc.sync.dma_start(wro[:], outw_d[o, tt * 128:(tt + 1) * 128, :])
                            prod = work.tile([128, E], f32, tag="prod", bufs=2)
                            nc.vector.tensor_mul(prod[:], xn, wro[:])
                            nc.vector.reduce_sum(acc[:, o:o + 1], prod[:],
                                                 axis=mybir.AxisListType.X)
                        nc.sync.dma_start(headp_d[tt], acc[:])
                    else:
                        for ec in range(4):
                            trp = ps_tr.tile([128, 128], f32, tag="trps")
                            nc.tensor.transpose(trp[:], xn[:, ec * 128:(ec + 1) * 128],
                                                ident[:])
                            nc.any.tensor_copy(xTc_sb[:, ec, tt * 128:(tt + 1) * 128],
                                               trp[:])

                if l < L - 1:
                    for ec in range(4):
                        nc.sync.dma_start(ag2_in[ec * 128:(ec + 1) * 128, :],
                                          xTc_sb[:, ec, :])
                    ag2_out = dram.tile([BS, E], bf16, tag="ag2_out", addr_space="Shared")
                    nc.gpsimd.collective_compute(
                        "AllGather", mybir.AluOpType.bypass,
                        replica_groups=[list(range(NC))],
                        ins=[ag2_in.opt()], outs=[ag2_out.opt()],
                    )
                    xsrc_ap = ag2_out[:]

    nc.compile()
    return nc


# ---------------------------------------------------------------------------
# host side
# ---------------------------------------------------------------------------
_STATE: dict = {}


def _pos_encoding_np():
    pos = np.arange(S, dtype=np.float32)[:, None]
    div = np.exp(np.arange(0, E, 2, dtype=np.float32) * (-np.log(10000.0) / E))
    pe = np.zeros((S, E), np.float32)
    pe[:, 0::2] = np.sin(pos * div)
    pe[:, 1::2] = np.cos(pos * div)
    return pe


def _bf(x):
    return np.ascontiguousarray(np.asarray(x, np.float32).astype(ml_dtypes.bfloat16))


def _f32(x):
    return np.ascontiguousarray(np.asarray(x, np.float32))


def kernel(inputs, emb, ln_w, ln_b, q_w, q_b, k_w, k_b, v_w, v_b,
           fc1_w, fc1_b, fc2_w, fc2_b, out_w, out_b):
    idx = np.asarray(inputs)
    emb = _f32(emb)
    x0 = emb[idx.reshape(-1)] + np.tile(_pos_encoding_np(), (B, 1))  # [BS, E]
    # AG2-compatible layout: row g*512 + e, col = local token within chunk g
    x0_arr = np.ascontiguousarray(
        x0.reshape(NC, CHUNK, E).transpose(0, 2, 1).reshape(BS, E))

    if "nc" not in _STATE:
        _STATE["nc"] = build_nc()
    nc = _STATE["nc"]

    q_w, k_w, v_w = _f32(q_w), _f32(k_w), _f32(v_w)
    fc1_w, fc2_w = _f32(fc1_w), _f32(fc2_w)
    out_w = _f32(out_w)
    Wr = out_w.reshape(S, E, OUT)

    x0_bf = _bf(x0_arr)
    fc1_pack = _bf(fc1_w.reshape(4, 128, HID).transpose(1, 0, 2))
    fc1b_pack = _f32(np.asarray(fc1_b, np.float32).reshape(HID // 128, 128).T)
    fc2_pack = _bf(fc2_w.reshape(HID // 128, 128, E).transpose(1, 0, 2))

    in_maps = []
    for c in range(NC):
        hs = slice(c * DH, (c + 1) * DH)
        lg = c % 4  # within-batch chunk -> rows of out_w (shared across batches)
        qk = np.concatenate([q_w[:, hs], k_w[:, hs]], axis=1)  # [E, 128]
        in_maps.append({
            "xsrc0": x0_bf,
            "qkw": _bf(qk.reshape(4, 128, 128).transpose(1, 0, 2)),
            "vw": _bf(v_w[:, hs].reshape(4, 128, DH).transpose(1, 0, 2)),
            "qkb": _f32(np.concatenate([np.asarray(q_b, np.float32)[hs],
                                        np.asarray(k_b, np.float32)[hs]])[:, None]),
            "vb": _f32(np.asarray(v_b, np.float32)[hs][:, None]),
            "fc1w": fc1_pack,
            "fc1b": fc1b_pack,
            "fc2w": fc2_pack,
            "fc2b": _f32(fc2_b),
            "lnw": _f32(ln_w),
            "lnb": _f32(ln_b),
            "outw": _f32(Wr[lg * CHUNK:(lg + 1) * CHUNK].transpose(2, 0, 1)),
        })

    res = run_bass_kernel_spmd(nc, in_maps, core_ids=list(range(NC)))
    _STATE["last_results"] = res

    out = np.zeros((B, OUT), np.float64)
    for c in range(NC):
        out[c // 4] += res.results[c]["headp"].sum(axis=(0, 1), dtype=np.float64)
    out += np.asarray(out_b, np.float32)[None, :].astype(np.float64)
    return out.astype(np.float32)
